# revision 17
# baseline (speedup 1.0000x reference)
"""TRN2 Bass kernel for nn_MAD_4612794876395 (retrieval_knn).

Math: with dist = softmax_k(-||pos_d - pos_r||) and sum_k dist = 1, the
reference output collapses to
    out[b,c] = wmem@adapt_w + adapt_b + wdiff@field_b.reshape(H,C)
             + sum_h wdiff[b,h] * (date@field_w)[b, h*C+c]
where wdiff[b,h] = sum_k dist[b,k]*diff[b,k,h].  The dominant term is the
137 GFLOP date@field_w product, computed on 8 NeuronCores tensor-parallel
over field_w's 65536 columns (64 h-values per core) as fp16 matmuls at
1 row/cycle.

The h-contraction (one multiply-add per matmul output element) is the
hard part: per-partition-scalar ops force 128-wide tiles and the three
elementwise engines together cannot sustain 4 such ops per 853ns matmul
chain.  Instead field_w columns are reordered c-major/h-minor on the
host so each PSUM tile is [128b, 8c x 64h], and a custom DVE op
(out = running sum of in0*in1, one elem/cycle) computes weighted prefix
sums in a single 512-wide pass; the 8 segment ends are DMA-extracted
and the host finishes with a cheap difference.  Small terms are host
numpy.
"""
import sys

sys.path.insert(0, "/opt/trn_rl_repo")

import numpy as np

N_DATA, F, H, C, K, B = 100000, 512, 512, 128, 8, 2048
NCORES = 8
HSH = H // NCORES          # 64 h-values per core
SH = HSH * C               # 8192 field_w cols per core
P = 128
NB = B // P                # 16 b-tiles
NS = SH // 512             # 16 col-slices of 512 (8 c-segments x 64 h)
CS = SH // HSH             # 128 c-values per core... (= C)

_NC = None
_LAST_IN_MAPS = None

_WSUM_SHAS = {"v3": "b3fc3e78a862b7eb", "v4": "bc6a002865d48b97"}


def _register_wsum():
    """Register the weighted-prefix-sum custom DVE op (idempotent)."""
    from concourse import dve_ops
    from concourse.dve_spec import Spec, Src0, Src1, scan, AluOp

    name = "ANT_WSUM_SCAN"
    for op in dve_ops.OPS:
        if op.name == name:
            return op

    def ref(in0, in1, s0, s1, imm2):
        p0 = in0.astype(np.float32).reshape(in0.shape[0], -1)
        p1 = in1.astype(np.float32).reshape(in1.shape[0], -1)
        return np.cumsum(p0 * p1, axis=-1).reshape(in0.shape)

    spec = Spec(body=scan(AluOp.ADD, Src0 * Src1), reference=ref)
    op = dve_ops.DveOp(name, spec, subdim=False, uops_sha=dict(_WSUM_SHAS))
    dve_ops.OPS.append(op)
    dve_ops._SUB_OPCODE_FOR_NAME[name] = (
        max(dve_ops._SUB_OPCODE_FOR_NAME.values()) + 1)
    assert dve_ops._SUB_OPCODE_FOR_NAME[name] < 0x20
    return op


def _build():
    import concourse.bass as bass
    import concourse.mybir as mybir
    import concourse.tile as tile
    from concourse import bacc

    wsum = _register_wsum()

    nc = bacc.Bacc(None, target_bir_lowering=False, debug=False)
    # dateT tiled into contiguous column-quarters: row block (q*4+fc)*128
    # holds dateT[fc*128:(fc+1)*128, q*512:(q+1)*512]
    dtq = nc.dram_tensor("dtq", [16 * P, 512], mybir.dt.float16,
                         kind="ExternalInput")
    # wdiff b-tiled: [p, t, h] = wdiff[t*128+p, h] -> one contiguous DMA
    wds = nc.dram_tensor("wds", [P, NB * HSH], mybir.dt.float32,
                         kind="ExternalInput")
    fw = nc.dram_tensor("fw", [NS * 4 * P, 512], mybir.dt.float16,
                        kind="ExternalInput")
    # prefix-sum segment ends; host differences them into per-c sums
    ends = nc.dram_tensor("ends", [B, C], mybir.dt.float32,
                          kind="ExternalOutput")

    with tile.TileContext(nc) as tc:
        with (
            tc.tile_pool(name="const", bufs=1) as cp,
            tc.tile_pool(name="fwp", bufs=4) as fwp,
            tc.tile_pool(name="wgp", bufs=8) as wgp,
            tc.tile_pool(name="ps", bufs=8, space="PSUM") as ps,
        ):
            # resident fp16 dateT f-chunks, wdiff rows, and SBUF staging
            # tiles for the segment ends
            dr = [cp.tile([P, B], mybir.dt.float16, name=f"d{fc}")
                  for fc in range(4)]
            wrall = cp.tile([P, NB, HSH], mybir.dt.float32, name="wrall")
            es = [cp.tile([P, C], mybir.dt.float32, name=f"es{t}")
                  for t in range(NB)]
            # per-b-tile wdiff rows repeated 8x via a stride-0 middle dim
            wrb = []
            for t in range(NB):
                s = wrall[:, t:t + 1, :]
                wrb.append(bass.AP(s.tensor, s.offset,
                                   [s.ap[0], [0, 8], s.ap[-1]]))

            def dma_dr_quarter(q):
                for fc in range(4):
                    nc.sync.dma_start(
                        dr[fc][:, q * 512:(q + 1) * 512],
                        dtq[(q * 4 + fc) * P:(q * 4 + fc + 1) * P, :])

            for n in range(NS):
                if n == 0:
                    # startup priority order: quarter-0 of dateT, slice-0
                    # fw tiles (in the loop below), then wdiff + the rest
                    dma_dr_quarter(0)
                fwr = []
                for fc in range(4):
                    f_t = fwp.tile([P, 512], mybir.dt.float16, name="f_t",
                                   tag=f"f{fc}")
                    nc.sync.dma_start(
                        f_t[:], fw[(n * 4 + fc) * P:(n * 4 + fc + 1) * P, :])
                    fwr.append(f_t)
                if n == 0:
                    # separate queue: keeps the PE's batched startup wait
                    # (sync-queue sem) scoped to dr-q0 + slice-0 fw only
                    nc.scalar.dma_start(wrall[:], wds[:, :])
                    for q in range(1, 4):
                        dma_dr_quarter(q)
                for t in range(NB):
                    g = ps.tile([P, 512], mybir.dt.float32, name="g", tag="g")
                    for fc in range(4):
                        nc.tensor.matmul(g[:], dr[fc][:, t * P:(t + 1) * P],
                                         fwr[fc][:], start=(fc == 0),
                                         stop=(fc == 3))
                    # weighted prefix sum over the tile in one DVE pass
                    wg = wgp.tile([P, 8, HSH], mybir.dt.float32, name="wg",
                                  tag="wg")
                    nc.vector._custom_dve(wsum, out=wg[:], in0=g[:],
                                          in1=wrb[t])
                    # stage segment ends in SBUF (GPSIMD is otherwise idle)
                    nc.gpsimd.tensor_copy(es[t][:, n * 8:(n + 1) * 8],
                                          wg[:, :, HSH - 1:HSH])
                    if n == NS - 1:
                        nc.sync.dma_start(ends[t * P:(t + 1) * P, :],
                                          es[t][:])
    nc.finalize()
    return nc


def kernel(idx, date, train_dates, mem, train_nns, pos_w, pos_b, field_w,
           field_b, adapt_w, adapt_b):
    global _NC, _LAST_IN_MAPS
    from concourse.bass_utils import run_bass_kernel_spmd

    idx = np.asarray(idx)
    date = np.asarray(date, dtype=np.float32)
    train_dates = np.asarray(train_dates, dtype=np.float32)
    mem = np.asarray(mem, dtype=np.float32)
    train_nns = np.asarray(train_nns)
    pos_w = np.asarray(pos_w, dtype=np.float32)
    pos_b = np.asarray(pos_b, dtype=np.float32)
    field_w = np.asarray(field_w, dtype=np.float32)
    field_b = np.asarray(field_b, dtype=np.float32)
    adapt_w = np.asarray(adapt_w, dtype=np.float32)
    adapt_b = np.asarray(adapt_b, dtype=np.float32)

    # ---- host phase 1 (small): dist, wdiff, const terms ----
    refs = train_nns[idx]                                   # [B, K]
    pos_d = date @ pos_w + pos_b                            # [B, H]
    pos_r = (train_dates[refs.reshape(-1)] @ pos_w + pos_b).reshape(B, K, H)
    diff = pos_d[:, None, :] - pos_r                        # [B, K, H]
    norm = np.sqrt((diff * diff).sum(-1))                   # [B, K]
    m = norm.min(axis=1, keepdims=True)
    e = np.exp(m - norm)
    dist = e / e.sum(axis=1, keepdims=True)                 # [B, K]
    wdiff = np.einsum("bk,bkh->bh", dist, diff).astype(np.float32)
    wmem = np.einsum("bk,bkc->bc", dist, mem[refs]).astype(np.float32)
    const = wmem @ adapt_w + adapt_b + wdiff @ field_b.reshape(H, C)

    # ---- device phase 2: grad-term, TP over the 65536 dim ----
    if _NC is None:
        _NC = _build()
    dateT16 = date.T.astype(np.float16)                     # [F, B]
    # contiguous column-quarters: row block (q*4+fc)*128 holds
    # dateT[fc*128:(fc+1)*128, q*512:(q+1)*512]
    dtq = np.ascontiguousarray(
        dateT16.reshape(4, P, 4, 512).transpose(2, 0, 1, 3).reshape(-1, 512))
    fw3 = field_w.reshape(F, H, C)                          # [f, h, c]
    in_maps = []
    for i in range(NCORES):
        # c-major/h-minor columns for this core's h range
        fw2 = np.ascontiguousarray(
            fw3[:, i * HSH:(i + 1) * HSH, :].transpose(0, 2, 1)
        ).reshape(F, SH).astype(np.float16)                 # col = c*64 + h
        # tile layout: row block (n*4+fc)*128 holds fw2[fc*128:(fc+1)*128,
        # n*512:(n+1)*512] so every SBUF tile DMA is one contiguous read
        fwt = np.ascontiguousarray(
            fw2.reshape(4, P, NS, 512).transpose(2, 0, 1, 3).reshape(-1, 512))
        # wdiff b-tiled [p, t, h] = wdiff[t*128+p, i*64+h]
        wdt = np.ascontiguousarray(
            wdiff[:, i * HSH:(i + 1) * HSH]
            .reshape(NB, P, HSH).transpose(1, 0, 2).reshape(P, NB * HSH))
        in_maps.append({
            "dtq": dtq,
            "wds": wdt,
            "fw": fwt,
        })
    _LAST_IN_MAPS = in_maps
    res = run_bass_kernel_spmd(_NC, in_maps, core_ids=list(range(NCORES)))
    grad_term = np.zeros((B, C), dtype=np.float32)
    for i in range(NCORES):
        e8 = res.results[i]["ends"].reshape(B, NS, 8)
        grad_term += np.diff(e8, axis=2, prepend=0.0).reshape(B, C)
    return (const + grad_term).astype(np.float32)


def run_device(trace=False):
    """Re-run the device phase on the last inputs (test.py profiling)."""
    from concourse.bass_utils import run_bass_kernel_spmd
    assert _NC is not None and _LAST_IN_MAPS is not None
    return run_bass_kernel_spmd(_NC, _LAST_IN_MAPS,
                                core_ids=list(range(NCORES)), trace=trace)


# revision 18
# speedup vs baseline: 1.0088x; 1.0088x over previous
"""TRN2 Bass kernel for nn_MAD_4612794876395 (retrieval_knn).

Math: with dist = softmax_k(-||pos_d - pos_r||) and sum_k dist = 1, the
reference output collapses to
    out[b,c] = wmem@adapt_w + adapt_b + wdiff@field_b.reshape(H,C)
             + sum_h wdiff[b,h] * (date@field_w)[b, h*C+c]
where wdiff[b,h] = sum_k dist[b,k]*diff[b,k,h].  The dominant term is the
137 GFLOP date@field_w product, computed on 8 NeuronCores tensor-parallel
over field_w's 65536 columns (64 h-values per core) as fp16 matmuls at
1 row/cycle.

The h-contraction (one multiply-add per matmul output element) is the
hard part: per-partition-scalar ops force 128-wide tiles and the three
elementwise engines together cannot sustain 4 such ops per 853ns matmul
chain.  Instead field_w columns are reordered c-major/h-minor on the
host so each PSUM tile is [128b, 8c x 64h], and a custom DVE op
(out = running sum of in0*in1, one elem/cycle) computes weighted prefix
sums in a single 512-wide pass; the 8 segment ends are DMA-extracted
and the host finishes with a cheap difference.  Small terms are host
numpy.
"""
import sys

sys.path.insert(0, "/opt/trn_rl_repo")

import numpy as np

N_DATA, F, H, C, K, B = 100000, 512, 512, 128, 8, 2048
NCORES = 8
HSH = H // NCORES          # 64 h-values per core
SH = HSH * C               # 8192 field_w cols per core
P = 128
NB = B // P                # 16 b-tiles
NS = SH // 512             # 16 col-slices of 512 (8 c-segments x 64 h)
CS = SH // HSH             # 128 c-values per core... (= C)

_NC = None
_LAST_IN_MAPS = None

_WSUM_SHAS = {"v3": "b3fc3e78a862b7eb", "v4": "bc6a002865d48b97"}


def _register_wsum():
    """Register the weighted-prefix-sum custom DVE op (idempotent)."""
    from concourse import dve_ops
    from concourse.dve_spec import Spec, Src0, Src1, scan, AluOp

    name = "ANT_WSUM_SCAN"
    for op in dve_ops.OPS:
        if op.name == name:
            return op

    def ref(in0, in1, s0, s1, imm2):
        p0 = in0.astype(np.float32).reshape(in0.shape[0], -1)
        p1 = in1.astype(np.float32).reshape(in1.shape[0], -1)
        return np.cumsum(p0 * p1, axis=-1).reshape(in0.shape)

    spec = Spec(body=scan(AluOp.ADD, Src0 * Src1), reference=ref)
    op = dve_ops.DveOp(name, spec, subdim=False, uops_sha=dict(_WSUM_SHAS))
    dve_ops.OPS.append(op)
    dve_ops._SUB_OPCODE_FOR_NAME[name] = (
        max(dve_ops._SUB_OPCODE_FOR_NAME.values()) + 1)
    assert dve_ops._SUB_OPCODE_FOR_NAME[name] < 0x20
    return op


def _build():
    import concourse.bass as bass
    import concourse.mybir as mybir
    import concourse.tile as tile
    from concourse import bacc

    wsum = _register_wsum()

    nc = bacc.Bacc(None, target_bir_lowering=False, debug=False)
    # dateT tiled into contiguous column-quarters: row block (q*4+fc)*128
    # holds dateT[fc*128:(fc+1)*128, q*512:(q+1)*512]
    dtq = nc.dram_tensor("dtq", [16 * P, 512], mybir.dt.float16,
                         kind="ExternalInput")
    # wdiff b-tiled: [p, t, h] = wdiff[t*128+p, h] -> one contiguous DMA
    wds = nc.dram_tensor("wds", [P, NB * HSH], mybir.dt.float32,
                         kind="ExternalInput")
    fw = nc.dram_tensor("fw", [NS * 4 * P, 512], mybir.dt.float16,
                        kind="ExternalInput")
    # prefix-sum segment ends; host differences them into per-c sums
    ends = nc.dram_tensor("ends", [B, C], mybir.dt.float32,
                          kind="ExternalOutput")

    with tile.TileContext(nc) as tc:
        with (
            tc.tile_pool(name="const", bufs=1) as cp,
            tc.tile_pool(name="fwp", bufs=4) as fwp,
            tc.tile_pool(name="wgp", bufs=8) as wgp,
            tc.tile_pool(name="ps", bufs=8, space="PSUM") as ps,
        ):
            # resident fp16 dateT f-chunks, wdiff rows, and SBUF staging
            # tiles for the segment ends
            dr = [cp.tile([P, B], mybir.dt.float16, name=f"d{fc}")
                  for fc in range(4)]
            wrall = cp.tile([P, NB, HSH], mybir.dt.float32, name="wrall")
            es = [cp.tile([P, C], mybir.dt.float32, name=f"es{t}")
                  for t in range(NB)]
            # per-b-tile wdiff rows repeated 8x via a stride-0 middle dim
            wrb = []
            for t in range(NB):
                s = wrall[:, t:t + 1, :]
                wrb.append(bass.AP(s.tensor, s.offset,
                                   [s.ap[0], [0, 8], s.ap[-1]]))

            def dma_dr_quarter(q):
                for fc in range(4):
                    nc.sync.dma_start(
                        dr[fc][:, q * 512:(q + 1) * 512],
                        dtq[(q * 4 + fc) * P:(q * 4 + fc + 1) * P, :])

            for n in range(NS):
                if n == 0:
                    # startup priority order: quarter-0 of dateT, slice-0
                    # fw tiles (in the loop below), then wdiff + the rest
                    dma_dr_quarter(0)
                fwr = []
                for fc in range(4):
                    f_t = fwp.tile([P, 512], mybir.dt.float16, name="f_t",
                                   tag=f"f{fc}")
                    nc.sync.dma_start(
                        f_t[:], fw[(n * 4 + fc) * P:(n * 4 + fc + 1) * P, :])
                    fwr.append(f_t)
                if n == 0:
                    nc.sync.dma_start(wrall[:], wds[:, :])
                    for q in range(1, 4):
                        dma_dr_quarter(q)
                for t in range(NB):
                    g = ps.tile([P, 512], mybir.dt.float32, name="g", tag="g")
                    for fc in range(4):
                        nc.tensor.matmul(g[:], dr[fc][:, t * P:(t + 1) * P],
                                         fwr[fc][:], start=(fc == 0),
                                         stop=(fc == 3))
                    # weighted prefix sum over the tile in one DVE pass
                    wg = wgp.tile([P, 8, HSH], mybir.dt.float32, name="wg",
                                  tag="wg")
                    nc.vector._custom_dve(wsum, out=wg[:], in0=g[:],
                                          in1=wrb[t])
                    # stage segment ends in SBUF (GPSIMD is otherwise idle)
                    nc.gpsimd.tensor_copy(es[t][:, n * 8:(n + 1) * 8],
                                          wg[:, :, HSH - 1:HSH])
                    if n == NS - 1:
                        nc.sync.dma_start(ends[t * P:(t + 1) * P, :],
                                          es[t][:])
    nc.finalize()
    return nc


def kernel(idx, date, train_dates, mem, train_nns, pos_w, pos_b, field_w,
           field_b, adapt_w, adapt_b):
    global _NC, _LAST_IN_MAPS
    from concourse.bass_utils import run_bass_kernel_spmd

    idx = np.asarray(idx)
    date = np.asarray(date, dtype=np.float32)
    train_dates = np.asarray(train_dates, dtype=np.float32)
    mem = np.asarray(mem, dtype=np.float32)
    train_nns = np.asarray(train_nns)
    pos_w = np.asarray(pos_w, dtype=np.float32)
    pos_b = np.asarray(pos_b, dtype=np.float32)
    field_w = np.asarray(field_w, dtype=np.float32)
    field_b = np.asarray(field_b, dtype=np.float32)
    adapt_w = np.asarray(adapt_w, dtype=np.float32)
    adapt_b = np.asarray(adapt_b, dtype=np.float32)

    # ---- host phase 1 (small): dist, wdiff, const terms ----
    refs = train_nns[idx]                                   # [B, K]
    pos_d = date @ pos_w + pos_b                            # [B, H]
    pos_r = (train_dates[refs.reshape(-1)] @ pos_w + pos_b).reshape(B, K, H)
    diff = pos_d[:, None, :] - pos_r                        # [B, K, H]
    norm = np.sqrt((diff * diff).sum(-1))                   # [B, K]
    m = norm.min(axis=1, keepdims=True)
    e = np.exp(m - norm)
    dist = e / e.sum(axis=1, keepdims=True)                 # [B, K]
    wdiff = np.einsum("bk,bkh->bh", dist, diff).astype(np.float32)
    wmem = np.einsum("bk,bkc->bc", dist, mem[refs]).astype(np.float32)
    const = wmem @ adapt_w + adapt_b + wdiff @ field_b.reshape(H, C)

    # ---- device phase 2: grad-term, TP over the 65536 dim ----
    if _NC is None:
        _NC = _build()
    dateT16 = date.T.astype(np.float16)                     # [F, B]
    # contiguous column-quarters: row block (q*4+fc)*128 holds
    # dateT[fc*128:(fc+1)*128, q*512:(q+1)*512]
    dtq = np.ascontiguousarray(
        dateT16.reshape(4, P, 4, 512).transpose(2, 0, 1, 3).reshape(-1, 512))
    fw3 = field_w.reshape(F, H, C)                          # [f, h, c]
    in_maps = []
    for i in range(NCORES):
        # c-major/h-minor columns for this core's h range
        fw2 = np.ascontiguousarray(
            fw3[:, i * HSH:(i + 1) * HSH, :].transpose(0, 2, 1)
        ).reshape(F, SH).astype(np.float16)                 # col = c*64 + h
        # tile layout: row block (n*4+fc)*128 holds fw2[fc*128:(fc+1)*128,
        # n*512:(n+1)*512] so every SBUF tile DMA is one contiguous read
        fwt = np.ascontiguousarray(
            fw2.reshape(4, P, NS, 512).transpose(2, 0, 1, 3).reshape(-1, 512))
        # wdiff b-tiled [p, t, h] = wdiff[t*128+p, i*64+h]
        wdt = np.ascontiguousarray(
            wdiff[:, i * HSH:(i + 1) * HSH]
            .reshape(NB, P, HSH).transpose(1, 0, 2).reshape(P, NB * HSH))
        in_maps.append({
            "dtq": dtq,
            "wds": wdt,
            "fw": fwt,
        })
    _LAST_IN_MAPS = in_maps
    res = run_bass_kernel_spmd(_NC, in_maps, core_ids=list(range(NCORES)))
    grad_term = np.zeros((B, C), dtype=np.float32)
    for i in range(NCORES):
        e8 = res.results[i]["ends"].reshape(B, NS, 8)
        grad_term += np.diff(e8, axis=2, prepend=0.0).reshape(B, C)
    return (const + grad_term).astype(np.float32)


def run_device(trace=False):
    """Re-run the device phase on the last inputs (test.py profiling)."""
    from concourse.bass_utils import run_bass_kernel_spmd
    assert _NC is not None and _LAST_IN_MAPS is not None
    return run_bass_kernel_spmd(_NC, _LAST_IN_MAPS,
                                core_ids=list(range(NCORES)), trace=trace)
